# revision 43
# baseline (speedup 1.0000x reference)
"""Causal self-attention on 8 Trainium2 NeuronCores.

Sharding: 8 cores = 4 batches x 2 head-groups (8 heads each).
Each core runs an identical SPMD program:
  - QKV projections for its head group (weights pre-transposed + bf16 on host)
  - causal attention computed in transposed-score layout S^T[s, t] so the
    AV matmul consumes P^T directly (no on-chip transposes at all)
  - softmax denominators come for free from a ones-column appended to V
  - row-sharded Wo projection produces a partial output; the two cores of a
    batch are summed on the host during unsharding.

Schedule: Q is kept in two zero-padded copies (even/odd head rows) so the
QK^T matmuls run with a full K=128 contraction - every matmul in the kernel
then uses the same PE array mode (no mode-switch drains), which lets the
builder interleave QK^T, AV (lagged 2 steps behind the exp) and
projection/output-projection "filler" matmuls into one dense PE stream that
stays busy while ScalarE computes the softmax exps.

B=4, T=2048, D=1024, H=16, dh=64.
"""

import numpy as np
import ml_dtypes

B, T, D = 4, 2048, 1024
P = 128
KD = D // P  # 8 contraction tiles for the input dim
HL = 8  # heads per core
HP = HL // 2  # head pairs per core (pair shares a 128-partition tile)
DH = 64
TCH = 512  # t-chunk (psum bank width in fp32)
NC4 = T // TCH  # 4 chunks
NTT = T // P  # 16 t-tiles
AVLAG = 2  # AV trails QK^T by this many s-tiles (hides exp latency)

_CACHE = {}


def _split_waits(nc, mybir, limit=1):
    """walrus in this container accepts at most one sem-wait per instruction;
    hoist extra waits onto preceding NoOps on the same engine."""
    cnt = 0
    for bb in nc.main_func.blocks:
        newlist = []
        for inst in bb.instructions:
            si = inst.sync_info
            if si is not None and len(si.on_wait) > limit:
                waits = list(si.on_wait)
                extra, keep = waits[:-limit], waits[-limit:]
                for w in extra:
                    cnt += 1
                    nop = mybir.InstNoOp(name=f"WSPLIT-{cnt}")
                    nop.engine = inst.engine
                    nop.sync_info = mybir.SyncInfo(on_wait=[w], on_update=[])
                    newlist.append(nop)
                inst.sync_info = mybir.SyncInfo(
                    on_wait=keep, on_update=list(si.on_update)
                )
            newlist.append(inst)
        bb.instructions[:] = newlist
    return cnt


def _build():
    if "nc" in _CACHE:
        return _CACHE["nc"]

    from contextlib import ExitStack

    import concourse.bass as bass
    import concourse.tile as tile
    from concourse import mybir

    f32 = mybir.dt.float32
    bf = mybir.dt.bfloat16
    Exp = mybir.ActivationFunctionType.Exp

    nc = bass.Bass()
    xT = nc.declare_dram_parameter("xT", [D, T], bf, isOutput=False)
    wq = nc.declare_dram_parameter("wq", [D, HL * DH], bf, isOutput=False)
    wk = nc.declare_dram_parameter("wk", [D, HL * DH], bf, isOutput=False)
    wv = nc.declare_dram_parameter("wv", [D, HL * DH], bf, isOutput=False)
    wo = nc.declare_dram_parameter("wo", [HL * DH, D], bf, isOutput=False)
    mk = nc.declare_dram_parameter("mask", [P, P], bf, isOutput=False)
    out = nc.declare_dram_parameter("out", [T, D], f32, isOutput=True)
    # DRAM bounce buffers for the softmax-denominator partition broadcast
    lds = [nc.dram_tensor(f"ld{i}", [T // 2], f32) for i in range(HP * NC4)]
    rds = [nc.dram_tensor(f"rd{i}", [T // 2], f32) for i in range(HP * NC4)]

    with tile.TileContext(nc) as tc, ExitStack() as ctx:
        psum = ctx.enter_context(tc.tile_pool(name="psum", bufs=1, space="PSUM"))
        per = ctx.enter_context(tc.tile_pool(name="per", bufs=1))

        wq_sb = per.tile([P, KD, HL * DH], bf)
        wk_sb = per.tile([P, KD, HL * DH], bf)
        wv_sb = per.tile([P, KD, HL * DH], bf)
        wo_sb = per.tile([P, HL * DH // P, D], bf)
        mk_sb = per.tile([P, P], bf)
        # Q^T in two zero-padded copies: qt0 has even-head rows (0:64) live,
        # qt1 odd-head rows (64:128); the other half stays zero so QK^T can
        # contract over all 128 partitions in the standard PE mode.
        # All of these are split into per-index tiles (instead of one big
        # tile with an extra axis) so Tile's dependency tracking stays exact.
        qt0_sb = {
            (m, cc): per.tile([P, TCH], bf, name=f"qt0_{m}_{cc}")
            for m in range(HP)
            for cc in range(NC4)
        }
        qt1_sb = {
            (m, cc): per.tile([P, TCH], bf, name=f"qt1_{m}_{cc}")
            for m in range(HP)
            for cc in range(NC4)
        }
        kt_sb = {
            (m, cc): per.tile([P, TCH], bf, name=f"kt_{m}_{cc}")
            for m in range(HP)
            for cc in range(NC4)
        }
        v_sb = [per.tile([P, HL, 66], bf, name=f"v_{tt}") for tt in range(NTT)]
        yt_sb = {
            (m, cc): per.tile([P, TCH], bf, name=f"yt_{m}_{cc}")
            for m in range(HP)
            for cc in range(NC4)
        }

        xT_sb = per.tile([P, KD, T], bf, name="xT_sb")

        # ---- loads + zero/one fills ----
        nc.sync.dma_start(out=mk_sb[:], in_=mk[:, :])
        for k in range(KD):
            nc.sync.dma_start(out=xT_sb[:, k, :], in_=xT[k * P : (k + 1) * P, :])
            nc.sync.dma_start(out=wq_sb[:, k, :], in_=wq[k * P : (k + 1) * P, :])
        for k in range(KD):
            nc.sync.dma_start(out=wk_sb[:, k, :], in_=wk[k * P : (k + 1) * P, :])
            nc.sync.dma_start(out=wv_sb[:, k, :], in_=wv[k * P : (k + 1) * P, :])
        for k in range(HL * DH // P):
            nc.sync.dma_start(out=wo_sb[:, k, :], in_=wo[k * P : (k + 1) * P, :])
        for tt in range(NTT):
            nc.vector.memset(v_sb[tt][:, :, 64:65], 1.0)
        for m in range(HP):
            for cc in range(NC4):
                nc.vector.memset(qt0_sb[(m, cc)][64:P, :], 0.0)
                nc.vector.memset(qt1_sb[(m, cc)][0:64, :], 0.0)

        pt_pool = ctx.enter_context(tc.tile_pool(name="ptp", bufs=8))
        ysb_pool = ctx.enter_context(tc.tile_pool(name="ysbp", bufs=2))
        sm_pool = ctx.enter_context(tc.tile_pool(name="smp", bufs=4))
        out_pool = ctx.enter_context(tc.tile_pool(name="outp", bufs=2))

        def gen_proj(cc):
            """QKV projections for chunk cc; yields after each matmul."""
            tsl = slice(cc * TCH, (cc + 1) * TCH)
            for m in range(HP):
                msl = slice(m * P, (m + 1) * P)
                pq = psum.tile([P, TCH], f32, tag="pp", bufs=2, name=f"pq{cc}_{m}")
                for k in range(KD):
                    nc.tensor.matmul(
                        out=pq[:],
                        lhsT=wq_sb[:, k, msl],
                        rhs=xT_sb[:, k, tsl],
                        start=(k == 0),
                        stop=(k == KD - 1),
                    )
                    yield
                nc.vector.tensor_copy(out=qt0_sb[(m, cc)][0:64, :], in_=pq[0:64, :])
                nc.vector.tensor_copy(out=qt1_sb[(m, cc)][64:P, :], in_=pq[64:P, :])
                pk = psum.tile([P, TCH], f32, tag="pp", bufs=2, name=f"pk{cc}_{m}")
                for k in range(KD):
                    nc.tensor.matmul(
                        out=pk[:],
                        lhsT=wk_sb[:, k, msl],
                        rhs=xT_sb[:, k, tsl],
                        start=(k == 0),
                        stop=(k == KD - 1),
                    )
                    yield
                nc.vector.tensor_copy(out=kt_sb[(m, cc)][:, :], in_=pk[:])
            for tt in range(4 * cc, 4 * cc + 4):
                pv = psum.tile([P, TCH], f32, tag="pp", bufs=2, name=f"pv{tt}")
                for k in range(KD):
                    nc.tensor.matmul(
                        out=pv[:],
                        lhsT=xT_sb[:, k, tt * P : (tt + 1) * P],
                        rhs=wv_sb[:, k, :],
                        start=(k == 0),
                        stop=(k == KD - 1),
                    )
                    yield
                nc.vector.tensor_copy(
                    out=v_sb[tt][:, :, 0:64],
                    in_=pv.rearrange("p (h d) -> p h d", h=HL),
                )

        def gen_oproj(chunks, korder=None, dma_eng=None):
            """Output projection for the given chunks; yields per matmul."""
            ks = list(korder or range(HL * DH // P))
            for c2 in chunks:
                for tt in range(4 * c2, 4 * c2 + 4):
                    ob = out_pool.tile([P, D], f32, tag="ob", name=f"ob{tt}")
                    for n2 in range(2):
                        po = psum.tile(
                            [P, TCH], f32, tag="pp", bufs=2, name=f"po{tt}_{n2}"
                        )
                        for i, k in enumerate(ks):
                            tl = tt - 4 * c2
                            nc.tensor.matmul(
                                out=po[:],
                                lhsT=yt_sb[(k, c2)][:, tl * P : (tl + 1) * P],
                                rhs=wo_sb[:, k, n2 * TCH : (n2 + 1) * TCH],
                                start=(i == 0),
                                stop=(i == len(ks) - 1),
                            )
                            yield
                        nc.vector.tensor_copy(
                            out=ob[:, n2 * TCH : (n2 + 1) * TCH], in_=po[:]
                        )
                    (dma_eng or nc.sync).dma_start(
                        out=out[tt * P : (tt + 1) * P, :], in_=ob[:]
                    )

        # projections for chunk 0 run unzipped up front (also warms the PE)
        for _ in gen_proj(0):
            pass

        # Global filler queue: a list of (tag, generator) consumed ~2 matmuls
        # per attention step; before attention chunk c its projections must be
        # fully emitted (Tile orders by program order), so drain through the
        # matching tag at each chunk start. O-proj generators are appended as
        # soon as their chunk's attention completes.
        fillq = [(("proj", cc), gen_proj(cc)) for cc in range(1, NC4)]

        def fill(n):
            done = 0
            while done < n and fillq:
                try:
                    next(fillq[0][1])
                    done += 1
                except StopIteration:
                    fillq.pop(0)
            return done

        def drain_through(tag):
            while fillq and any(t == tag for t, _ in fillq):
                try:
                    next(fillq[0][1])
                except StopIteration:
                    fillq.pop(0)

        FILL_PER_STEP = 2

        # ---- attention: per chunk, all head pairs, with filler zipped in ----
        for c in range(NC4):
            n_st = 4 * c + 4
            drain_through(("proj", c))

            # last chunk: run hp=3 first so the final O-proj (k emitted in the
            # same rotated order) is never blocked on the last norm chain
            hporder = [3, 0, 1, 2] if c == NC4 - 1 else list(range(HP))
            for hp in hporder:
                pts = {}
                psys = {}

                def emit_av(st, hp=hp, pts=pts, psys=psys, n_st=n_st):
                    pt, lo = pts[st]
                    for par in (0, 1):
                        if st == 0:
                            psys[par] = psum.tile(
                                [65, TCH], f32, tag="py", bufs=2, name=f"psy{par}"
                            )
                        nc.tensor.matmul(
                            out=psys[par][:, lo:TCH],
                            lhsT=v_sb[st][:, 2 * hp + par, 0:65],
                            rhs=pt[:, par, lo:TCH],
                            start=(st == 0),
                            stop=(st == n_st - 1),
                        )

                for st in range(n_st):
                    kd = st - 4 * c  # >=0 on causal-diagonal s-tiles
                    lo = max(kd, 0) * P
                    pss = psum.tile([P, 2 * TCH], f32, tag="ps2", bufs=2, name="pss")
                    for par, qt in ((0, qt0_sb), (1, qt1_sb)):
                        nc.tensor.matmul(
                            out=pss[:, par * TCH + lo : (par + 1) * TCH],
                            lhsT=kt_sb[(hp, st // 4)][:, (st % 4) * P : (st % 4 + 1) * P],
                            rhs=qt[(hp, c)][:, lo:TCH],
                            start=True,
                            stop=True,
                        )
                    pt = pt_pool.tile([P, 2, TCH], bf, tag="pt", name="pt")
                    nc.scalar.activation(
                        out=pt[:, :, lo:TCH],
                        in_=pss.rearrange("p (a b) -> p a b", a=2)[:, :, lo:TCH],
                        func=Exp,
                        scale=1.0 / np.sqrt(DH),
                    )
                    if kd >= 0:
                        for par in (0, 1):
                            nc.vector.tensor_mul(
                                pt[:, par, lo : lo + P],
                                pt[:, par, lo : lo + P],
                                mk_sb[:],
                            )
                    pts[st] = (pt, lo)
                    if st >= AVLAG:
                        emit_av(st - AVLAG)
                    # reserve remaining filler for the post-attention tail
                    # window (covers the last norm chain's DMA latency)
                    if not (c == NC4 - 1 and hp == hporder[-1]):
                        fill(FILL_PER_STEP)
                for st in range(n_st - AVLAG, n_st):
                    emit_av(st)

                # normalize: y^T = psy[0:64] / psy[64] (denominator row).
                # Copy the denominator row first so its DMA chain starts
                # before the big evict copies run.
                ysb = ysb_pool.tile([P, 2 * TCH], f32, tag="ysb", name="ysb")
                for par in (0, 1):
                    nc.vector.tensor_copy(
                        out=ysb[64:65, par * TCH : (par + 1) * TCH],
                        in_=psys[par][64:65, :],
                    )
                it = hp * NC4 + c
                ld, rd = lds[it], rds[it]
                nc.sync.dma_start(out=ld[:], in_=ysb[64:65, :])
                for par in (0, 1):
                    nc.vector.tensor_copy(
                        out=ysb[0:64, par * TCH : (par + 1) * TCH],
                        in_=psys[par][0:64, :],
                    )
                l128 = sm_pool.tile([P, 8], f32, tag="l128", name="l128")
                nc.sync.dma_start(
                    out=l128[:], in_=bass.AP(tensor=ld, offset=0, ap=[[8, P], [1, 8]])
                )
                r128 = sm_pool.tile([P, 8], f32, tag="r128", name="r128")
                nc.vector.reciprocal(out=r128[:], in_=l128[:])
                nc.sync.dma_start(
                    out=bass.AP(tensor=rd, offset=0, ap=[[8, P], [1, 8]]), in_=r128[:]
                )
                rb = sm_pool.tile([64, 2 * TCH], f32, tag="rb", name="rb")
                nc.sync.dma_start(
                    out=rb[:],
                    in_=bass.AP(tensor=rd, offset=0, ap=[[0, 64], [1, 2 * TCH]]),
                )
                for par in (0, 1):
                    rows = slice(64 * par, 64 * par + 64)
                    nc.vector.tensor_mul(
                        yt_sb[(hp, c)][rows, :],
                        ysb[0:64, par * TCH : (par + 1) * TCH],
                        rb[:, par * TCH : (par + 1) * TCH],
                    )
            # this chunk's output projection becomes available filler
            if c < NC4 - 1:
                fillq.append((("oproj", c), gen_oproj([c])))

        # drain remaining filler, then the last chunk's output projection
        while fill(64):
            pass
        for _ in gen_oproj([3], korder=[3, 0, 1, 2], dma_eng=nc.scalar):
            pass

    _split_waits(nc, mybir, 1)
    _CACHE["nc"] = nc
    return nc


def kernel(x, Wq, Wk, Wv, Wo):
    from concourse.bass_utils import run_bass_kernel_spmd

    nc = _build()
    bf16 = ml_dtypes.bfloat16

    band = np.tril(np.ones((P, P), np.float32)).T.astype(bf16)  # band[s,j]=s<=j
    xTs = [np.ascontiguousarray(x[b].T).astype(bf16) for b in range(B)]
    in_maps = []
    for c in range(8):
        b, hg = divmod(c, 2)
        sl = slice(512 * hg, 512 * hg + 512)
        in_maps.append(
            {
                "xT": xTs[b],
                "wq": np.ascontiguousarray(Wq[sl, :].T).astype(bf16),
                "wk": np.ascontiguousarray(Wk[sl, :].T).astype(bf16),
                "wv": np.ascontiguousarray(Wv[sl, :].T).astype(bf16),
                "wo": np.ascontiguousarray(Wo[:, sl].T).astype(bf16),
                "mask": band,
            }
        )
    res = run_bass_kernel_spmd(nc, in_maps, list(range(8)))
    _CACHE["exec_time_ns"] = res.exec_time_ns
    outp = np.empty((B, T, D), np.float32)
    for b in range(B):
        outp[b] = res.results[2 * b]["out"] + res.results[2 * b + 1]["out"]
    return outp
